# revision 3
# baseline (speedup 1.0000x reference)
"""Trainium2 Bass kernel v3: GQA attention + RoPE + block-diagonal causal
masking, sharded over 8 NeuronCores by KV head group (4 Q heads + 1 KV head
per core, both batches).

v3 design:
  - all matmul operands bf16 (full PE rate at any moving size); PSUM fp32.
  - attention at TQ=128 with all 4 heads in one matmul (moving = 4*cols);
    sub-diagonal tiles restrict columns to the first-doc range.
  - masking folded into the scores PSUM: diagonal tiles add a precomputed
    bias tile (0 / -30000) via an identity matmul; band-start tiles use the
    exp's per-partition bias operand.  No vector/pool ops between exp and PV.
  - one-tile lookahead: PV(kt) is emitted after scores(kt+1) so the Act exp
    overlaps PE work; o_proj and V-transposes are software-pipelined one
    step behind.
  - batched DMAs; x loads issued from the Pool sequencer with prefetch
    distance 2, y stores from SP.  y partials in bf16, summed on host.
"""
import sys
sys.path.insert(0, "/opt/trn_rl_repo")
import numpy as np

B, S, DIM = 2, 2048, 2048
NH, NKV, HD = 32, 8, 64
HPC = NH // 8            # 4 q-heads per core
MLOC = HPC * HD          # 256 local q dims
TQ = 128                 # attention query tile
NCORES = 8
NKC = DIM // 128         # 16 contraction chunks
NTC = S // 128           # 16 token tiles
SCALE = 1.0 / 8.0
NEG = -30000.0

_nc_cache = {}


def _schedule(doc_ids):
    """Per (batch, query-tile): band start tile t0, sub-diagonal column limit
    c1, band-start partition offset off1, and diagonal boundary offsets."""
    doc = np.asarray(doc_ids)
    sched = []
    for b in range(B):
        d = doc[b]
        change = np.empty(S, dtype=np.int64)
        change[0] = 0
        idx = np.arange(1, S)
        change[1:] = np.where(d[1:] != d[:-1], idx, 0)
        start_idx = np.maximum.accumulate(change)
        bounds = np.nonzero(change)[0]
        per_qc = []
        for qc in range(S // TQ):
            q0 = qc * TQ
            s1 = int(start_idx[q0])
            t0 = s1 // 128
            inner = [int(p) - q0 for p in bounds if q0 < p < q0 + TQ]
            c1 = inner[0] if inner else TQ
            per_qc.append({"t0": t0, "c1": c1, "off1": s1 - t0 * 128,
                           "bnds": tuple(inner)})
        sched.append(per_qc)
    return sched


def _build_nc(sched):
    import concourse.bacc as bacc
    import concourse.mybir as mybir
    import concourse.tile as tile
    from concourse.masks import make_identity

    F32, BF16 = mybir.dt.float32, mybir.dt.bfloat16
    Exp = mybir.ActivationFunctionType.Exp
    GE = mybir.AluOpType.is_ge

    nc = bacc.Bacc()
    xT = nc.dram_tensor("xT", (B, DIM, S), BF16, kind="ExternalInput")
    wq = nc.dram_tensor("wq", (DIM, MLOC), BF16, kind="ExternalInput")
    wkv = nc.dram_tensor("wkv", (DIM, 128), BF16, kind="ExternalInput")
    wo = nc.dram_tensor("wo", (MLOC, DIM), BF16, kind="ExternalInput")
    cos128 = nc.dram_tensor("cos128", (128, S), BF16, kind="ExternalInput")
    sin128 = nc.dram_tensor("sin128", (128, S), BF16, kind="ExternalInput")
    y = nc.dram_tensor("y", (B, S, DIM), BF16, kind="ExternalOutput")

    # which (b, qc) need a custom diagonal bias tile (doc boundary inside)
    need_bias = [(b, qc) for b in range(B) for qc in range(S // TQ)
                 if sched[b][qc]["bnds"]]
    # band-start exp bias columns, one per distinct off1 value
    offs = sorted({e["off1"] for sb in sched for e in sb if e["off1"] > 0})
    off_col = {o: i for i, o in enumerate(offs)}

    with tile.TileContext(nc) as tc:
        with (
            tc.tile_pool(name="const", bufs=1) as cst,
            tc.tile_pool(name="xt", bufs=3) as xtp,
            tc.tile_pool(name="big", bufs=2) as big,
            tc.tile_pool(name="rope", bufs=3) as rp,
            tc.tile_pool(name="pt", bufs=12) as ptp,
            tc.tile_pool(name="small", bufs=3) as sp,
            tc.tile_pool(name="ysb", bufs=3) as yp,
            tc.tile_pool(name="pp", bufs=2, space="PSUM") as pp,
            tc.tile_pool(name="pa", bufs=6, space="PSUM") as pa,
        ):
            # ---- weights / tables (SP seq); split so the first proj
            # matmuls start after ~64KB instead of the full 1MB, and keep
            # wo (not needed until the first o_proj) off the early BW.
            wq_sb = cst.tile([128, NKC, MLOC], BF16)
            wq_r = wq[:].rearrange("(kc p) m -> p kc m", p=128)
            for lo, hi in ((0, 2), (2, 6), (6, NKC)):
                nc.sync.dma_start(wq_sb[:, lo:hi, :], wq_r[:, lo:hi, :])
            wkv_sb = cst.tile([128, NKC, 128], BF16)
            wkv_r = wkv[:].rearrange("(kc p) m -> p kc m", p=128)
            for lo, hi in ((0, 2), (2, 6), (6, NKC)):
                nc.sync.dma_start(wkv_sb[:, lo:hi, :], wkv_r[:, lo:hi, :])
            cos_sb = cst.tile([128, S], BF16)
            nc.sync.dma_start(cos_sb[:], cos128[:])
            sin_sb = cst.tile([128, S], BF16)
            nc.sync.dma_start(sin_sb[:], sin128[:])
            wo_sb = cst.tile([128, 2, DIM], BF16)
            nc.sync.dma_start(wo_sb[:], wo[:].rearrange("(c p) m -> p c m", p=128))
            ident = cst.tile([64, 64], BF16)
            make_identity(nc, ident[:])
            id128 = cst.tile([128, 128], BF16)
            make_identity(nc, id128[:])

            # ---- x chunk prefetch machinery (Pool seq) ----
            chunks = [(b, tqi) for b in range(B) for tqi in range(4)]
            xt_tiles = {}

            def issue_xt(gi):
                bb, ti = chunks[gi]
                t = xtp.tile([128, NKC, 512], BF16, tag="xt", name=f"xt{bb}{ti}")
                xt_tiles[(bb, ti)] = t
                src = xT[bb].rearrange("(kc p) s -> p kc s", p=128)
                tsl = slice(ti * 512, ti * 512 + 512)
                if gi == 0:
                    for lo, hi in ((0, 2), (2, 6), (6, 11), (11, NKC)):
                        nc.gpsimd.dma_start(t[:, lo:hi, :], src[:, lo:hi, tsl])
                else:
                    nc.gpsimd.dma_start(t[:], src[:, :, tsl])

            issue_xt(0)
            issue_xt(1)

            # ---- mask bias tiles (Pool, built during b0 projections) ----
            causal_b = cst.tile([128, TQ, 4], BF16)
            diag_b = {}

            def build_bias():
                nc.gpsimd.memset(causal_b[:], 0.0)
                nc.gpsimd.affine_select(
                    out=causal_b[:], in_=causal_b[:], compare_op=GE, fill=NEG,
                    base=0, pattern=[[1, TQ], [0, 4]], channel_multiplier=-1)
                for (bb, qc) in need_bias:
                    t = cst.tile([128, TQ, 4], BF16, name=f"db{bb}{qc}")
                    nc.gpsimd.tensor_copy(t[:], causal_b[:])
                    for p in sched[bb][qc]["bnds"]:
                        nc.gpsimd.affine_select(
                            out=t[:, p:TQ, :], in_=t[:, p:TQ, :],
                            compare_op=GE, fill=NEG, base=-p,
                            pattern=[[0, TQ - p], [0, 4]], channel_multiplier=1)
                    diag_b[(bb, qc)] = t

            bandb = None
            if offs:
                bandb = cst.tile([128, len(offs)], F32)

            def build_bandb():
                nc.gpsimd.memset(bandb[:], 0.0)
                for o, i in off_col.items():
                    nc.gpsimd.affine_select(
                        out=bandb[:, i:i + 1], in_=bandb[:, i:i + 1],
                        compare_op=GE, fill=NEG, base=-o,
                        pattern=[[0, 1]], channel_multiplier=1)

            pend_tp = []        # pending V transposes: (gsb, base_kt)
            tp_n = [0]

            def flush_tp(vaug):
                while pend_tp:
                    gsb, bkt = pend_tp.pop(0)
                    for tc4 in range(4):
                        tp_n[0] += 1
                        ptr = pa.tile([128, 64], BF16, tag="pa",
                                      name=f"ptr{tp_n[0]}")
                        nc.tensor.transpose(
                            ptr[:], gsb[0:64, tc4 * 128:(tc4 + 1) * 128], ident[:])
                        nc.scalar.copy(vaug[:, bkt + tc4, 0:64], ptr[:])

            # ====== attention: batches interleaved, 3-stage pipeline ======
            # stage A(u): scores + bias + exp     (PE + Act)
            # stage B(u): PV accumulate + norm    (PE + DVE)
            # stage C(u): o_proj + y cast + DMA   (PE + Act/DVE + SP)
            st = {}

            def stage_a(b, qc):
                qr4, krt = qr4d[b], krtd[b]
                qr2 = qr4[:].rearrange("p s j -> p (s j)")
                q0 = qc * TQ
                ent = sched[b][qc]
                t0, c1, off1 = ent["t0"], ent["c1"], ent["off1"]
                pts = []
                for kt in range(t0, qc + 1):
                    diag = kt == qc
                    cols = TQ if diag else c1
                    s_ps = pa.tile([128, 4 * TQ], F32, tag="pa",
                                   name=f"sps{b}{qc}{kt}")
                    nc.tensor.matmul(
                        s_ps[:, 0:4 * cols], krt[:, kt * 128:(kt + 1) * 128],
                        qr2[:, 4 * q0:4 * (q0 + cols)], start=True, stop=not diag)
                    if diag:
                        db = diag_b.get((b, qc))
                        nc.tensor.matmul(
                            s_ps[:],
                            id128[:],
                            (db if db is not None else causal_b)[:]
                            .rearrange("p q j -> p (q j)"),
                            start=False, stop=True)
                    pt = ptp.tile([128, 4 * TQ], BF16, tag="pt")
                    bias = 0.0
                    if kt == t0 and off1 > 0:
                        bias = bandb[:, off_col[off1]:off_col[off1] + 1]
                    nc.scalar.activation(
                        pt[:, 0:4 * cols], s_ps[:, 0:4 * cols], Exp,
                        bias=bias, scale=SCALE)
                    pts.append((kt, pt))
                st[(b, qc)] = pts

            def stage_b(b, qc):
                vaug, or2t = vaugd[b], or2td[b]
                q0 = qc * TQ
                ent = sched[b][qc]
                t0, c1 = ent["t0"], ent["c1"]
                o_ps = pa.tile([128, 4 * TQ], F32, tag="pa", name=f"ops{b}{qc}")
                for kt, pt in st.pop((b, qc)):
                    if kt == qc:
                        if c1 < TQ and t0 < qc:
                            # bank was zeroed by the kt==t0 start=True; the
                            # [4*c1:] bytes are still pending-zero, so plain
                            # accumulation is correct for both regions.
                            nc.tensor.matmul(
                                o_ps[:, 0:4 * c1], vaug[:, kt, :],
                                pt[:, 0:4 * c1], start=False, stop=False)
                            nc.tensor.matmul(
                                o_ps[:, 4 * c1:], vaug[:, kt, :],
                                pt[:, 4 * c1:], start=False, stop=True)
                        else:
                            nc.tensor.matmul(
                                o_ps[:], vaug[:, kt, :], pt[:],
                                start=(t0 == qc), stop=True)
                    else:
                        nc.tensor.matmul(
                            o_ps[:, 0:4 * c1], vaug[:, kt, :], pt[:, 0:4 * c1],
                            start=(kt == t0), stop=False)
                zb = sp.tile([64, 4 * TQ], F32, tag="zb")
                with nc.allow_low_precision(reason="softmax reciprocal"):
                    nc.vector.reciprocal(zb[:], o_ps[64:128, :])
                o3 = o_ps[:].rearrange("p (q j) -> p q j", j=4)
                z3 = zb[:].rearrange("p (q j) -> p q j", j=4)
                for j in range(4):
                    nc.vector.tensor_mul(
                        or2t[j // 2][(j % 2) * 64:(j % 2) * 64 + 64, q0:q0 + TQ],
                        o3[0:64, :, j], z3[:, :, j])

            def stage_c(b, qc):
                or2t = or2td[b]
                q0 = qc * TQ
                y_row = yp.tile([128, DIM], BF16, tag="ysb")
                for mc in range(4):
                    y_ps = pa.tile([128, 512], F32, tag="pa",
                                   name=f"yps{b}{qc}{mc}")
                    for hp in range(2):
                        nc.tensor.matmul(
                            y_ps[:], or2t[hp][:, q0:q0 + TQ],
                            wo_sb[:, hp, mc * 512:(mc + 1) * 512],
                            start=(hp == 0), stop=(hp == 1))
                    if mc < 2:
                        nc.scalar.copy(y_row[:, mc * 512:(mc + 1) * 512], y_ps[:])
                    else:
                        nc.vector.tensor_copy(
                            y_row[:, mc * 512:(mc + 1) * 512], y_ps[:])
                nc.sync.dma_start(y[b, q0:q0 + TQ, :], y_row[:])

            units = [(b, qc) for b in range(B) for qc in range(S // TQ)]
            NU = len(units)

            def pipe_step(i):
                if i < NU:
                    stage_a(*units[i])
                if 1 <= i < NU + 1:
                    stage_b(*units[i - 1])
                if i >= 2:
                    stage_c(*units[i - 2])

            next_i = [0]

            def pipe_to(k):
                while next_i[0] < min(k, NU + 2):
                    pipe_step(next_i[0])
                    next_i[0] += 1

            qr4d, krtd, vaugd, or2td = {}, {}, {}, {}
            for b in range(B):
                qr4 = qr4d[b] = big.tile([64, S, 4], BF16, tag="qr4", name=f"qr4{b}")
                krt = krtd[b] = big.tile([64, S], BF16, tag="krt", name=f"krt{b}")
                vaug = vaugd[b] = big.tile([128, NTC, 128], BF16, tag="vaug",
                                           name=f"vaug{b}")
                or2td[b] = [big.tile([128, S], BF16, tag=f"or2t{m}", name=f"or2t{m}{b}")
                            for m in range(2)]
                nc.vector.memset(vaug[:, :, 64:128], 1.0)

                # ================= projections + rope =================
                for tqi in range(4):
                    gi = b * 4 + tqi
                    if gi + 2 < len(chunks):
                        issue_xt(gi + 2)
                    t0c = tqi * 512
                    tsl = slice(t0c, t0c + 512)
                    xt_sb = xt_tiles[(b, tqi)]
                    gps_d = {}
                    if b == 0 and tqi == 0:
                        # interleave q0/q1 kc-wise so each arriving x piece
                        # feeds two groups; uses exactly the 2 pp banks.
                        for w in ("q0", "q1"):
                            gps_d[w] = pp.tile([128, 512], F32, tag="pp",
                                               name=f"gps{b}{tqi}{w}")
                        for kc in range(NKC):
                            for wi, w in enumerate(("q0", "q1")):
                                nc.tensor.matmul(
                                    gps_d[w][:],
                                    wq_sb[:, kc, wi * 128:(wi + 1) * 128],
                                    xt_sb[:, kc, :],
                                    start=(kc == 0), stop=(kc == NKC - 1))
                    for what in ("q0", "q1", "kv"):
                        gps = gps_d.get(what)
                        if gps is None:
                            gps = pp.tile([128, 512], F32, tag="pp",
                                          name=f"gps{b}{tqi}{what}")
                            for kc in range(NKC):
                                if what == "q0":
                                    lhs = wq_sb[:, kc, 0:128]
                                elif what == "q1":
                                    lhs = wq_sb[:, kc, 128:256]
                                else:
                                    lhs = wkv_sb[:, kc, :]
                                nc.tensor.matmul(
                                    gps[:], lhs, xt_sb[:, kc, :],
                                    start=(kc == 0), stop=(kc == NKC - 1))
                        gsb = rp.tile([128, 512], BF16, tag="gsb")
                        nc.scalar.copy(gsb[:], gps[:])
                        if what in ("q0", "q1"):
                            j0 = 0 if what == "q0" else 2
                            u = rp.tile([128, 512], BF16, tag="ra")
                            for blk in (0, 64):
                                nc.vector.tensor_mul(
                                    u[blk:blk + 32], gsb[blk + 32:blk + 64],
                                    sin_sb[blk + 32:blk + 64, tsl])
                                nc.vector.tensor_mul(
                                    u[blk + 32:blk + 64], gsb[blk:blk + 32],
                                    sin_sb[blk:blk + 32, tsl])
                            t2 = rp.tile([128, 512], BF16, tag="rb")
                            nc.vector.tensor_mul(t2[:], gsb[:], cos_sb[:, tsl])
                            nc.vector.tensor_add(
                                qr4[0:64, tsl, j0], u[0:64], t2[0:64])
                            nc.vector.tensor_add(
                                qr4[0:64, tsl, j0 + 1], u[64:128], t2[64:128])
                        else:
                            u = rp.tile([128, 512], BF16, tag="ra")
                            nc.vector.tensor_mul(
                                u[64:96], gsb[96:128], sin_sb[96:128, tsl])
                            nc.vector.tensor_mul(
                                u[96:128], gsb[64:96], sin_sb[64:96, tsl])
                            t2 = rp.tile([128, 512], BF16, tag="rb")
                            nc.vector.tensor_mul(
                                t2[64:128], gsb[64:128], cos_sb[64:128, tsl])
                            nc.vector.tensor_add(
                                krt[0:64, tsl], u[64:128], t2[64:128])
                            flush_tp(vaug)
                            pend_tp.append((gsb, t0c // 128))
                        if b == 1:
                            g1 = tqi * 3 + ("q0", "q1", "kv").index(what) + 1
                            cap = 16 + 4 * max(0, g1 // 3 - 1)
                            pipe_to(min(12 + (g1 * 20 + 11) // 12, cap))
                    if b == 0 and tqi == 0:
                        if bandb is not None:
                            build_bandb()
                        build_bias()
                    if b == 0 and tqi >= 1:
                        pipe_to(4 * tqi)
                flush_tp(vaug)

            for i in range(next_i[0], NU + 2):
                pipe_step(i)

    nc.finalize()
    return nc


def _prep_inputs(x, rope_cos, rope_sin, doc_ids, Wq, Wk, Wv, Wo):
    import ml_dtypes
    BF = ml_dtypes.bfloat16
    x = np.asarray(x, np.float32)
    xT = np.ascontiguousarray(x.transpose(0, 2, 1)).astype(BF)
    cosT = np.asarray(rope_cos, np.float32).T          # (32, S)
    sinT = np.asarray(rope_sin, np.float32).T
    cos128 = np.tile(np.concatenate([cosT, cosT], 0), (2, 1)).astype(BF)
    sin128 = np.tile(np.concatenate([sinT, -sinT], 0), (2, 1)).astype(BF)
    sched = _schedule(doc_ids)
    Wq = np.asarray(Wq, np.float32)
    Wk = np.asarray(Wk, np.float32)
    Wv = np.asarray(Wv, np.float32)
    Wo = np.asarray(Wo, np.float32)
    in_maps = []
    for c in range(NCORES):
        wq_c = np.ascontiguousarray(Wq[c * MLOC:(c + 1) * MLOC].T).astype(BF)
        wk_c = Wk[c * HD:(c + 1) * HD].T
        wv_c = Wv[c * HD:(c + 1) * HD].T
        wkv_c = np.ascontiguousarray(np.concatenate([wv_c, wk_c], 1)).astype(BF)
        wo_c = np.ascontiguousarray(Wo[:, c * MLOC:(c + 1) * MLOC].T).astype(BF)
        in_maps.append({
            "xT": xT, "wq": wq_c, "wkv": wkv_c, "wo": wo_c,
            "cos128": cos128, "sin128": sin128,
        })
    return sched, in_maps


def _sched_key(sched):
    return tuple(
        tuple((e["t0"], e["c1"], e["off1"], e["bnds"]) for e in sb)
        for sb in sched)


def kernel(x, rope_cos, rope_sin, doc_ids, Wq, Wk, Wv, Wo):
    from concourse.bass_utils import run_bass_kernel_spmd
    sched, in_maps = _prep_inputs(
        x, rope_cos, rope_sin, doc_ids, Wq, Wk, Wv, Wo)
    key = _sched_key(sched)
    nc = _nc_cache.get(key)
    if nc is None:
        nc = _build_nc(sched)
        _nc_cache[key] = nc
    res = run_bass_kernel_spmd(nc, in_maps, core_ids=list(range(NCORES)))
    y = np.zeros((B, S, DIM), np.float32)
    for c in range(NCORES):
        y += res.results[c]["y"].astype(np.float32)
    return y
